# revision 31
# baseline (speedup 1.0000x reference)
"""GCN (5-layer message passing) on 8 Trainium2 NeuronCores.

Sharding: nodes (and their in-edges, partitioned by edge_row) are sharded
across the 8 cores.  Per layer:
  support = h @ W        -> per-shard matmul on PE (h kept transposed [f, n])
  AllGather(support)     -> full bf16 table in each core's DRAM
  msgs = support[col]    -> dma_gather (per-edge 256B rows, 4 col-windows
                            because dma_gather indices are int16)
  agg[row] += val*msgs   -> PE matmuls: agg_T[f, n] += msgs[slots, f]^T @ Sel
                            with edge_val folded into host-built bf16 Sel
  h' = relu(agg + b)     -> fused in the PSUM->SBUF copy on ScalarE (features
                            on partitions => bias is per-partition)

All shapes are static and identical across cores: nodes are greedily packed
into groups of <=128 nodes with <=SLOTW gather slots per col-window; the
node space is padded (holes) so every group occupies a fixed 128-row block.
"""

import os
import sys
import time

import numpy as np

sys.path.insert(0, "/opt/trn_rl_repo")

import ml_dtypes

BF16 = ml_dtypes.bfloat16

# ---------------- problem constants (hardcoded per the contract) -----------
N_NODES = 100000
N_EDGES = 1600000
NFEAT = 256
NHID = 128
NCORES = 8
SHARD = N_NODES // NCORES  # 12500
P = 128

SLOTW = 512          # gather slots per (group, window); 4 chunks of 128
CHW = SLOTW // P     # chunks per (group, window) = 4
GIB = 4              # groups per batch in phase B

_CACHE = {}


# ======================= host preprocessing ================================

def _pack_groups(cnt_w):
    """cnt_w: [n_nodes_local, NW] per-window slot counts.  Greedy pack
    consecutive nodes into groups with <=128 nodes and <=SLOTW slots per
    window.  Returns g_of, ofs_of (arrays) and ngroups."""
    n = cnt_w.shape[0]
    g_of = np.zeros(n, np.int32)
    ofs_of = np.zeros(n, np.int32)
    g = 0
    nodes_in_g = 0
    slots = np.zeros(cnt_w.shape[1], np.int64)
    for i in range(n):
        c = cnt_w[i]
        assert c.max() <= SLOTW, "node in-degree exceeds group budget"
        if nodes_in_g >= P or (slots + c).max() > SLOTW:
            g += 1
            nodes_in_g = 0
            slots[:] = 0
        g_of[i] = g
        ofs_of[i] = nodes_in_g
        nodes_in_g += 1
        slots += c
    return g_of, ofs_of, g + 1


def preprocess(edge_row, edge_col, edge_val):
    """Build all per-core static arrays.  Returns (meta, per_core_inputs)."""
    edge_row = np.asarray(edge_row).astype(np.int64)
    edge_col = np.asarray(edge_col).astype(np.int64)
    edge_val = np.asarray(edge_val).astype(np.float32)

    core_of_edge = edge_row // SHARD
    per_core = []
    for c in range(NCORES):
        m = core_of_edge == c
        per_core.append((edge_row[m] - c * SHARD, edge_col[m], edge_val[m]))

    # windows: 2 shards per window -> 4 windows (fits int16 as long as G<=127)
    NW = 4
    WPG = NCORES // NW  # shards per window

    # ---- group packing per core (based on local rows' per-window counts)
    g_of = []
    ofs_of = []
    ngroups = []
    for c in range(NCORES):
        er, ec, _ = per_core[c]
        ww = (ec // SHARD) // WPG
        cnt = np.zeros((SHARD, NW), np.int64)
        np.add.at(cnt, (er, ww), 1)
        gf, of, ng = _pack_groups(cnt)
        g_of.append(gf)
        ofs_of.append(of)
        ngroups.append(ng)
    G = max(ngroups)
    G = -(-G // GIB) * GIB  # round up to batch multiple
    assert WPG * P * G <= 32767, f"int16 window overflow: G={G}"
    NPAD = P * G            # padded nodes per shard
    TBL = NCORES * NPAD     # padded table rows
    WROWS = WPG * NPAD      # rows per window

    # padded local id of every (core, local node)
    padded_local = [P * g_of[c] + ofs_of[c] for c in range(NCORES)]

    meta = dict(NW=NW, WPG=WPG, G=G, NPAD=NPAD, TBL=TBL, WROWS=WROWS)

    # ---- per-core slot/Sel/idx construction
    inputs = []
    for c in range(NCORES):
        er, ec, ev = per_core[c]
        pc = ec // SHARD                    # col's core
        w = pc // WPG                       # window
        # padded col id within its core:
        pcol = np.empty(len(ec), np.int64)
        for cc in range(NCORES):
            mm = pc == cc
            if mm.any():
                pcol[mm] = padded_local[cc][ec[mm] - cc * SHARD]
        idx16 = (pc % WPG) * NPAD + pcol    # gather index within window
        assert idx16.max() < 32768

        g = g_of[c][er]                     # row's group
        mofs = ofs_of[c][er]                # row's offset within group

        # slot within (g, w) segment: sort by (segment, col idx) — the idx
        # subsort makes each segment's gather descriptors hit increasing
        # HBM addresses (row-buffer locality), then cumcount
        seg = g * NW + w
        order = np.lexsort((idx16, seg))
        seg_sorted = seg[order]
        diff = np.r_[True, seg_sorted[1:] != seg_sorted[:-1]]
        run_starts = np.flatnonzero(diff)
        run_ids = np.cumsum(diff) - 1
        k_within = np.arange(len(seg_sorted)) - run_starts[run_ids]
        assert k_within.max() < SLOTW

        # window-stream slot: group g occupies [SLOTW*g, SLOTW*(g+1)) in its window
        slot = SLOTW * g[order] + k_within  # slot within window stream
        wo = w[order]

        # idx arrays [NW, 128, SLOTW*G/16] int16 (16-row wrap, replicated x8)
        ncol = SLOTW * G // 16
        idx_arr = np.zeros((NW, 16, ncol), np.int16)
        idx_arr[wo, slot % 16, slot // 16] = idx16[order].astype(np.int16)
        idx_arr = np.tile(idx_arr, (1, 8, 1))  # replicate to 128 partitions

        # Sel blob: [G, 128, NW*CHW*128] bf16 (per group: partition=slot%128,
        # free = (w*CHW + j)*128 + m)
        sel = np.zeros((G, P, NW * CHW * P), np.float32)
        j = k_within // P
        s128 = k_within % P
        free = (wo * CHW + j) * P + mofs[order]
        sel[g[order], s128, free] = ev[order]
        sel = sel.astype(BF16)

        inputs.append(dict(idx=idx_arr, sel=sel))

    meta["padded_local"] = padded_local
    meta["g_of"] = g_of
    return meta, inputs


def build_xt(x, meta):
    """Per-core transposed, padded, bf16 node features [G, KT, 128, 128]."""
    NPAD = meta["NPAD"]
    G = meta["G"]
    KT = NFEAT // P
    out = []
    for c in range(NCORES):
        xs = x[c * SHARD:(c + 1) * SHARD].astype(np.float32)
        xt = np.zeros((NFEAT, NPAD), np.float32)
        xt[:, meta["padded_local"][c]] = xs.T
        xt = xt.reshape(KT, P, G, P).transpose(2, 0, 1, 3)  # [G, KT, 128, 128]
        out.append(np.ascontiguousarray(xt).astype(BF16))
    return out


# ======================= device program ====================================

def build_program(meta, ablate=(), nlayers=5):
    from concourse import bacc, mybir
    import concourse.tile as tile
    ablate = set(ablate)

    NW, G, NPAD, TBL, WROWS = meta["NW"], meta["G"], meta["NPAD"], meta["TBL"], meta["WROWS"]
    KT = NFEAT // P
    NB = G // GIB
    f32 = mybir.dt.float32
    bf16 = mybir.dt.bfloat16
    i16 = mybir.dt.int16

    nc = bacc.Bacc("TRN2", target_bir_lowering=False, debug=False,
                   num_devices=NCORES)

    xt_d = nc.dram_tensor("xt", [G, KT, P, P], bf16, kind="ExternalInput").ap()
    idx_d = nc.dram_tensor("idx", [NW, P, SLOTW * G // 16], i16,
                           kind="ExternalInput").ap()
    sel_d = nc.dram_tensor("sel", [G, P, NW * CHW * P], bf16,
                           kind="ExternalInput").ap()
    w1_d = nc.dram_tensor("w1", [KT, P, NHID], bf16, kind="ExternalInput").ap()
    w2_d = nc.dram_tensor("w2", [P, NHID], bf16, kind="ExternalInput").ap()
    b1_d = nc.dram_tensor("b1", [P, 1], f32, kind="ExternalInput").ap()
    b2_d = nc.dram_tensor("b2", [P, 1], f32, kind="ExternalInput").ap()
    out_d = nc.dram_tensor("out", [P, NPAD], f32, kind="ExternalOutput").ap()

    bounce = nc.dram_tensor("bounce", [NPAD, NHID], bf16)
    table = nc.dram_tensor("table", [TBL, NHID], bf16, addr_space="Shared")

    if "min" in ablate:
        # minimal program touching every input once: measures the per-exec
        # floor of a NEFF with the same bound tensors
        with tile.TileContext(nc) as tc:
            with tc.tile_pool(name="m", bufs=2) as mp:
                acc = mp.tile([P, P], f32, name="acc")
                t1 = mp.tile([P, P], bf16, name="t1")
                nc.sync.dma_start(out=t1[:], in_=xt_d[0, 0])
                t2 = mp.tile([P, P], bf16, name="t2")
                nc.sync.dma_start(out=t2[:], in_=sel_d[0][:, :P])
                t3 = mp.tile([P, 32], i16, name="t3")
                nc.sync.dma_start(out=t3[:], in_=idx_d[0][:, :32])
                t4 = mp.tile([P, NHID], bf16, name="t4")
                nc.sync.dma_start(out=t4[:], in_=w1_d[0])
                t5 = mp.tile([P, NHID], bf16, name="t5")
                nc.sync.dma_start(out=t5[:], in_=w2_d[:])
                t6 = mp.tile([P, 1], f32, name="t6")
                nc.sync.dma_start(out=t6[:], in_=b1_d[:])
                t7 = mp.tile([P, 1], f32, name="t7")
                nc.sync.dma_start(out=t7[:], in_=b2_d[:])
                nc.vector.tensor_copy(out=acc[:], in_=t1[:])
                for g in range(G):
                    nc.sync.dma_start(out=out_d[:, g * P:(g + 1) * P],
                                      in_=acc[:])
        nc.compile()
        return nc

    with tile.TileContext(nc) as tc:
        with tc.tile_pool(name="const", bufs=1) as constp, \
             tc.tile_pool(name="ht", bufs=1) as htp, \
             tc.tile_pool(name="lhs", bufs=4) as lhsp, \
             tc.tile_pool(name="sup", bufs=4) as supp, \
             tc.tile_pool(name="msgs", bufs=3 * NW, space="SBUF") as msgsp, \
             tc.tile_pool(name="selp", bufs=3) as selp, \
             tc.tile_pool(name="outp", bufs=4) as outp, \
             tc.tile_pool(name="psum", bufs=2, space="PSUM") as psp, \
             tc.tile_pool(name="psumb", bufs=2, space="PSUM") as psbp:

            # ---- resident constants
            w1_t = constp.tile([P, KT, NHID], bf16, tag="w1")
            for k in range(KT):
                nc.sync.dma_start(out=w1_t[:, k, :], in_=w1_d[k])
            w2_t = constp.tile([P, NHID], bf16, tag="w2")
            nc.sync.dma_start(out=w2_t[:], in_=w2_d[:])
            b1_t = constp.tile([P, 1], f32, tag="b1")
            nc.sync.dma_start(out=b1_t[:], in_=b1_d[:])
            b2_t = constp.tile([P, 1], f32, tag="b2")
            nc.sync.dma_start(out=b2_t[:], in_=b2_d[:])
            ncol = SLOTW * G // 16
            idx_t = constp.tile([P, NW, ncol], i16, tag="idx")
            for w in range(NW):
                nc.sync.dma_start(out=idx_t[:, w, :], in_=idx_d[w])

            # h_T buffer [128 f, NPAD] bf16 (written by phase B, read by A)
            ht = htp.tile([P, NPAD], bf16, tag="ht")
            if "phaseB" in ablate:
                nc.vector.memset(ht[:], 0.0)

            # ablation helpers: persistent stand-in tiles so nothing is
            # read-without-write when a stage is skipped
            sel_static = None
            if "sel" in ablate:
                sel_static = constp.tile([P, GIB * NW * CHW * P], bf16,
                                         tag="selstat")
                for gi in range(GIB):
                    nc.scalar.dma_start(
                        out=sel_static[:, gi * (NW * CHW * P):
                                       (gi + 1) * (NW * CHW * P)],
                        in_=sel_d[gi])
            mt_static = None
            if "gather" in ablate:
                mt_static = [constp.tile([P, GIB * CHW, NHID], bf16,
                                         name=f"mstat{w}", tag=f"mstat{w}")
                             for w in range(NW)]
                for w in range(NW):
                    nc.vector.memset(mt_static[w][:], 0.0)

            def emit_A_group(g, layer):
                """support(layer) for group g -> bounce rows [128g, 128g+128)."""
                ps = psp.tile([P, NHID], f32, name="psA", tag="psA",
                              space="PSUM")
                if layer == 0:
                    for k in range(KT):
                        lt = lhsp.tile([P, P], bf16, name="lt", tag="lhs")
                        nc.sync.dma_start(out=lt[:], in_=xt_d[g, k])
                        nc.tensor.matmul(out=ps[:], lhsT=lt[:],
                                         rhs=w1_t[:, k, :],
                                         start=(k == 0), stop=(k == KT - 1))
                else:
                    nc.tensor.matmul(out=ps[:],
                                     lhsT=ht[:, g * P:(g + 1) * P],
                                     rhs=w2_t[:], start=True, stop=True)
                st = supp.tile([P, NHID], bf16, name="st", tag="sup")
                nc.vector.tensor_copy(out=st[:], in_=ps[:])
                nc.sync.dma_start(out=bounce[g * P:(g + 1) * P, :], in_=st[:])

            def emit_AG():
                if "ag" in ablate:
                    nc.sync.dma_start(out=table[:NPAD, :], in_=bounce[:])
                else:
                    nc.gpsimd.collective_compute(
                        "AllGather", mybir.AluOpType.bypass,
                        replica_groups=[list(range(NCORES))],
                        ins=[bounce[:]], outs=[table[:]],
                    )

            def emit_B_batch(layer, b):
                last = layer == nlayers - 1
                b_t = b1_t if layer == 0 else b2_t
                mt = []
                for w in range(NW):
                    if mt_static is not None:
                        mt.append(mt_static[w])
                        continue
                    t = msgsp.tile([P, GIB * CHW, NHID], bf16, name="mtile",
                                   tag="msgs")
                    nc.gpsimd.dma_gather(
                        out_ap=t[:],
                        in_ap=table[w * WROWS:(w + 1) * WROWS, :],
                        idxs_ap=idx_t[:, w, b * (GIB * SLOTW // 16):
                                      (b + 1) * (GIB * SLOTW // 16)],
                        num_idxs=GIB * SLOTW,
                        num_idxs_reg=GIB * SLOTW,
                        elem_size=NHID,
                        single_packet=False,
                    )
                    mt.append(t)
                if sel_static is not None:
                    selt = sel_static
                else:
                    selt = selp.tile([P, GIB * NW * CHW * P], bf16,
                                     name="selt", tag="sel")
                    for gi in range(GIB):
                        nc.scalar.dma_start(
                            out=selt[:, gi * (NW * CHW * P):
                                     (gi + 1) * (NW * CHW * P)],
                            in_=sel_d[b * GIB + gi],
                        )
                pb = psbp.tile([P, GIB * P], f32, name="pb", tag="psB",
                               space="PSUM")
                for gi in range(GIB):
                    for w in range(NW):
                        for j in range(CHW):
                            if "mm" in ablate and not (w == 0 and j == 0):
                                continue
                            nc.tensor.matmul(
                                out=pb[:, gi * P:(gi + 1) * P],
                                lhsT=mt[w][:, gi * CHW + j, :],
                                rhs=selt[:, (gi * NW * CHW + w * CHW + j) * P:
                                         (gi * NW * CHW + w * CHW + j + 1) * P],
                                start=(w == 0 and j == 0),
                                stop=(w == 0 and j == 0) if "mm" in ablate
                                else (w == NW - 1 and j == CHW - 1),
                            )
                for gi in range(GIB):
                    g = b * GIB + gi
                    if last:
                        ot = outp.tile([P, P], f32, name="ot", tag="out")
                        nc.scalar.activation(
                            out=ot[:], in_=pb[:, gi * P:(gi + 1) * P],
                            func=mybir.ActivationFunctionType.Relu,
                            bias=b_t[:], scale=1.0)
                        nc.sync.dma_start(
                            out=out_d[:, g * P:(g + 1) * P], in_=ot[:])
                    else:
                        nc.scalar.activation(
                            out=ht[:, g * P:(g + 1) * P],
                            in_=pb[:, gi * P:(gi + 1) * P],
                            func=mybir.ActivationFunctionType.Relu,
                            bias=b_t[:], scale=1.0)

            # software-pipelined schedule: A(l+1) for a batch's groups is
            # emitted right after B(l) finishes that batch, so the next
            # layer's support/AG overlaps the current gather stream
            for g in range(G):
                emit_A_group(g, 0)
            emit_AG()
            for layer in range(nlayers):
                skipB = "phaseB" in ablate
                for b in range(NB):
                    if not skipB:
                        emit_B_batch(layer, b)
                    if layer < nlayers - 1:
                        for gi in range(GIB):
                            emit_A_group(b * GIB + gi, layer + 1)
                if layer < nlayers - 1:
                    emit_AG()

    nc.compile()
    return nc


# ======================= runner ============================================

class Runner:
    """Persistent PJRT runner (keeps the jitted executable + device inputs)."""

    def __init__(self, nc, n_cores=NCORES):
        import jax
        import jax.numpy as jnp
        from jax.sharding import Mesh, PartitionSpec, NamedSharding
        from jax.experimental.shard_map import shard_map
        from concourse import bass2jax, mybir

        bass2jax.install_neuronx_cc_hook()
        self.jax = jax
        self.nc = nc

        partition_name = (nc.partition_id_tensor.name
                          if nc.partition_id_tensor else None)
        in_names, out_names, out_avals, zero_outs = [], [], [], []
        for alloc in nc.m.functions[0].allocations:
            if not isinstance(alloc, mybir.MemoryLocationSet):
                continue
            name = alloc.memorylocations[0].name
            if alloc.kind == "ExternalInput":
                if name != partition_name:
                    in_names.append(name)
            elif alloc.kind == "ExternalOutput":
                out_names.append(name)
                shape = tuple(alloc.tensor_shape)
                dtype = mybir.dt.np(alloc.dtype)
                out_avals.append(jax.core.ShapedArray(shape, dtype))
                zero_outs.append(np.zeros(shape, dtype))
        self.in_names = list(in_names)
        self.out_names = out_names
        self.out_avals = out_avals
        n_params = len(in_names)
        all_in_names = in_names + out_names
        if partition_name is not None:
            all_in_names.append(partition_name)

        def _body(*args):
            operands = list(args)
            if partition_name is not None:
                operands.append(bass2jax.partition_id_tensor())
            outs = bass2jax._bass_exec_p.bind(
                *operands,
                out_avals=tuple(out_avals),
                in_names=tuple(all_in_names),
                out_names=tuple(out_names),
                lowering_input_output_aliases=(),
                sim_require_finite=True,
                sim_require_nnan=True,
                nc=nc,
            )
            return tuple(outs)

        devices = jax.devices()[:n_cores]
        self.mesh = Mesh(np.asarray(devices), ("core",))
        spec = PartitionSpec("core")
        self.sharding = NamedSharding(self.mesh, spec)
        in_specs = (spec,) * (n_params + len(out_names))
        out_specs = (spec,) * len(out_names)
        self.fn = jax.jit(
            shard_map(_body, mesh=self.mesh, in_specs=in_specs,
                      out_specs=out_specs, check_rep=False),
            keep_unused=True,
        )
        self.zero_outs = [
            jax.device_put(
                np.zeros((n_cores * z.shape[0], *z.shape[1:]), z.dtype),
                self.sharding)
            for z in zero_outs
        ]
        self.n_cores = n_cores

    def place(self, in_maps):
        concat = [
            np.concatenate([np.asarray(in_maps[c][n])
                            for c in range(self.n_cores)], axis=0)
            for n in self.in_names
        ]
        return [self.jax.device_put(a, self.sharding) for a in concat]

    def run(self, dev_in):
        outs = self.fn(*dev_in, *self.zero_outs)
        return outs

    def results(self, outs):
        res = []
        for c in range(self.n_cores):
            d = {}
            for i, n in enumerate(self.out_names):
                full = np.asarray(outs[i])
                per = full.reshape(self.n_cores, *self.out_avals[i].shape)
                d[n] = per[c]
            res.append(d)
        return res

    def time_ns(self, dev_in, iters=10, warmup=2):
        """Pipelined timing: dispatch K calls async, block once.  Amortizes
        the ~90 ms axon round-trip; per-call overhead ~2.3 ms remains."""
        for _ in range(warmup):
            self.jax.block_until_ready(self.fn(*dev_in, *self.zero_outs))
        K = 32
        best = float("inf")
        for _ in range(iters):
            t0 = time.perf_counter_ns()
            outs = [self.fn(*dev_in, *self.zero_outs) for _ in range(K)]
            self.jax.block_until_ready(outs)
            best = min(best, (time.perf_counter_ns() - t0) / K)
        return int(best)


# ======================= top-level entry ===================================

def _get_compiled(edge_row, edge_col, edge_val):
    key = (int(np.asarray(edge_row[:64]).sum()),
           int(np.asarray(edge_col[:64]).sum()), len(edge_row))
    hit = _CACHE.get(key)
    if hit is not None:
        return hit
    meta, per_core = preprocess(edge_row, edge_col, edge_val)
    nc = build_program(meta)
    runner = Runner(nc)
    _CACHE[key] = (meta, per_core, runner)
    return _CACHE[key]


def _make_in_maps(meta, per_core, x, W1, b1, W2, b2):
    xt = build_xt(np.asarray(x), meta)
    KT = NFEAT // P
    w1 = np.ascontiguousarray(
        np.asarray(W1, np.float32).reshape(KT, P, NHID)).astype(BF16)
    w2 = np.asarray(W2, np.float32).astype(BF16)
    b1v = np.asarray(b1, np.float32).reshape(P, 1)
    b2v = np.asarray(b2, np.float32).reshape(P, 1)
    maps = []
    for c in range(NCORES):
        maps.append(dict(
            xt=xt[c], idx=per_core[c]["idx"], sel=per_core[c]["sel"],
            w1=w1, w2=w2, b1=b1v, b2=b2v,
        ))
    return maps


def _assemble(meta, results):
    g_of = meta["g_of"]
    out = np.empty((N_NODES, NHID), np.float32)
    for c in range(NCORES):
        ht = results[c]["out"]             # [128 f, NPAD]
        h = ht.T                           # [NPAD, 128]
        pl = meta["padded_local"][c]
        out[c * SHARD:(c + 1) * SHARD] = h[pl]
    return out


def kernel(**inputs):
    edge_row = inputs["edge_row"]
    edge_col = inputs["edge_col"]
    edge_val = inputs["edge_val"]
    meta, per_core, runner = _get_compiled(edge_row, edge_col, edge_val)
    maps = _make_in_maps(meta, per_core, inputs["x"], inputs["W1"],
                         inputs["b1"], inputs["W2"], inputs["b2"])
    dev_in = runner.place(maps)
    outs = runner.run(dev_in)
    results = runner.results(outs)
    return _assemble(meta, results)
